# revision 21
# baseline (speedup 1.0000x reference)
"""ExtractSearchWindows Trainium2 kernel (8 NeuronCores, Bass/Tile).

out[b, h, w, dy*cv+dx, ky*8+kx] = uint8(P[b, h+off+dy+ky, w+off+dx+kx])
with P = zero-pad(inputs[:, 0], 7) and off = 3 - search_range.

The output (196.6 MB u8) is a pure byte-replication of a tiny input, so
the kernel is bound by per-core DMA-engine write bandwidth (~425 GB/s
across 16 engines; ~26.6 GB/s/engine for descriptors >= 4 KB, less for
small ones).  Work is sharded over (b, h): each of the 8 cores produces
48 output rows as 384 segments (segment = 40-pixel row chunk) in 3
tiles of 128 partitions.

Device-side expansion: strided uint32 DVE tensor_copies read host-
prepared byte-shifted sub-rows S[seg][v][u][j] (v = dy+ky source row,
u = phi+dx byte shift, j = 4a+4kxp+beta addressing pixel w = 4a+phi,
kx = 4*kxp+beta) and scatter them into out-staging tiles that DMA out
with large contiguous descriptors.

Pipeline fill: a small fast-start slice S0a is DMA'd first so the DVE
starts ~1 us earlier; pixels 0-11 of tile 0 drain via two dy-sliced
blocks (640/960 B descriptors, ~0.73x engine rate -- paid while the
engines would otherwise idle); everything later uses w-chunks with
19.2-32 KB descriptors at full rate, sized so the engines never
starve once the first block lands.
"""
import numpy as np

K = 8
MAX_SR = 3
B, H, W = 2, 192, 320
TP = MAX_SR + K // 2          # 7 pad per side
PW = W + 2 * TP               # 334
NCORES = 8
ROWS_PER_CORE = (B * H) // NCORES   # 48
WSEG = 40
NWSEG = W // WSEG             # 8
NSEG = ROWS_PER_CORE * NWSEG  # 384
NTILE = NSEG // 128           # 3

# sr=2 geometry
CV = 5
OSEG = WSEG * CV * CV * K * K   # 64000 output bytes per segment
PIXB = CV * CV * K * K          # 1600 output bytes per pixel
PIXW = PIXB // 4                # 400 u32 per pixel
DW = CV * K * K // 4            # 80 u32 per (pixel, dy)

NV = 12                       # source rows per segment (CV-1+K)
NU = 8                        # byte shifts u = phi+dx
NJ = 44                       # shifted sub-row bytes
SEGB = NV * NU * NJ           # 4224 S bytes per segment
A_NV, A_NJ = 12, 16           # fast-start slice: all v, j<=15 (a<=1)
A_B = A_NV * NU * A_NJ        # 1536
RJ = 56                       # compact row bytes (covers u+j <= 50)
RB = NV * RJ                  # 672 compact bytes per segment

# persistent SBUF layout (u8 offsets)
S0A_OFF = 0
S_OFF = A_B                   # S tiles at S_OFF + t*SEGB
R12_OFF = S_OFF + NTILE * SEGB
PERS_B = R12_OFF + 2 * RB

import os
SPLIT_QUEUES = os.environ.get("ESW_SPLIT_QUEUES", "0") == "1"

_PROG_CACHE = {}


def _make_host_arrays(x, sr):
    """x: (B,1,H,W) f32 -> per-core dict of host-prepped u8 arrays."""
    off = MAX_SR - sr
    P = np.pad(x[:, 0], ((0, 0), (TP, TP), (TP, TP))).astype(np.uint8)
    cores = []
    st = np.lib.stride_tricks.as_strided
    for c in range(NCORES):
        b = (c * ROWS_PER_CORE) // H
        h0 = (c * ROWS_PER_CORE) % H
        flat = np.ascontiguousarray(P[b]).reshape(-1)
        base = (h0 + off) * PW + off
        # S: tile-0 segments fully shifted: (r, s, v, u, j)
        s = st(flat[base:], shape=(16, NWSEG, NV, NU, NJ),
               strides=(PW, WSEG, PW, 1, 1))
        s = np.ascontiguousarray(s).reshape(128, SEGB)
        # S0a: fast-start slice of tile 0 (all v, j<16)
        s0a = st(flat[base:], shape=(16, NWSEG, A_NV, NU, A_NJ),
                 strides=(PW, WSEG, PW, 1, 1))
        s0a = np.ascontiguousarray(s0a).reshape(128, A_B)
        # R12: compact un-shifted rows for tiles 1,2: (t, r, s, v, j)
        r12 = st(flat[base + 16 * PW:], shape=(2, 16, NWSEG, NV, RJ),
                 strides=(16 * PW, PW, WSEG, PW, 1))
        r12 = np.ascontiguousarray(r12.transpose(1, 2, 0, 3, 4)) \
            .reshape(128, 2 * RB)
        cores.append({"s0a": s0a, "s": s, "r12": r12})
    return cores


def _build_program(sr):
    import concourse.bass as bass
    import concourse.bacc as bacc
    import concourse.mybir as mybir
    from concourse import tile

    u8 = mybir.dt.uint8
    u16 = mybir.dt.uint16
    u32 = mybir.dt.uint32
    nc = bacc.Bacc("TRN2", debug=False)
    s0a_in = nc.declare_dram_parameter("s0a", [128, A_B], u8, isOutput=False)
    s_in = nc.declare_dram_parameter("s", [128, SEGB], u8, isOutput=False)
    r12_in = nc.declare_dram_parameter("r12", [128, 2 * RB], u8,
                                       isOutput=False)
    out = nc.declare_dram_parameter("out", [NSEG * OSEG], u8, isOutput=True)

    with tile.TileContext(nc) as tc:
        with tc.tile_pool(name="spool", bufs=1) as sp, \
             tc.tile_pool(name="tpool", bufs=1) as tp:
            PS = sp.tile([128, PERS_B], u8)
            p8 = PS[:]
            p16 = PS[:].bitcast(u16)
            p32 = PS[:].bitcast(u32)
            PP8, PP16, PP32 = PERS_B, PERS_B // 2, PERS_B // 4

            # host data in, latency-critical first, all on the SP queue
            nc.sync.dma_start(PS[:, S0A_OFF:S0A_OFF + A_B], s0a_in[:, :])
            nc.sync.dma_start(PS[:, R12_OFF:R12_OFF + 2 * RB], r12_in[:, :])
            nc.sync.dma_start(PS[:, S_OFF:S_OFF + SEGB], s_in[:, :])

            def build_s(t, parts):
                """Shift compact rows R into S[t]: S[t][v][u][j] =
                R[t][v][u+j].  u%4==0 as u32, u%2==0 as u16 on DVE;
                odd u as u8 on the Activation engine."""
                rb8 = R12_OFF + (t - 1) * RB
                s8 = S_OFF + t * SEGB
                for u in range(NU):
                    if u % 2 == 0 and parts == "even":
                        if u % 4 == 0:
                            src = bass.AP(p32.tensor, rb8 // 4 + u // 4,
                                          [[PP32, 128], [RJ // 4, NV],
                                           [1, NJ // 4]])
                            dst = bass.AP(p32.tensor,
                                          s8 // 4 + u * (NJ // 4),
                                          [[PP32, 128], [NU * NJ // 4, NV],
                                           [1, NJ // 4]])
                            nc.vector.tensor_copy(dst, src)
                        else:
                            src = bass.AP(p16.tensor, rb8 // 2 + u // 2,
                                          [[PP16, 128], [RJ // 2, NV],
                                           [1, NJ // 2]])
                            dst = bass.AP(p16.tensor,
                                          s8 // 2 + u * (NJ // 2),
                                          [[PP16, 128], [NU * NJ // 2, NV],
                                           [1, NJ // 2]])
                            nc.vector.tensor_copy(dst, src)
                    elif u % 2 == 1 and parts == "odd":
                        src = bass.AP(p8.tensor, rb8 + u,
                                      [[PP8, 128], [RJ, NV], [1, NJ]])
                        dst = bass.AP(p8.tensor, s8 + u * NJ,
                                      [[PP8, 128], [NU * NJ, NV], [1, NJ]])
                        nc.scalar.copy(dst, src)

            # odd-byte shifts on the otherwise-idle Activation engine
            build_s(1, "odd")
            build_s(2, "odd")

            def expand(s_off32, src_st, T, t_pitch32, pix_w32, dys, dy0,
                       a0, an):
                """DVE scatter block: one copy per (dy in dys, phi 0..3).

                Reads S at u32 offset s_off32 (+ dy*sv + phi*su + a*sa),
                writes staging tile T laid out [pixel][dy-dy0][dx][ky][kx]
                with pix_w32 u32 per pixel.
                """
                sv, su, sa = src_st
                t32 = T[:].bitcast(u32)
                for dy in dys:
                    for phi in range(4):
                        src = bass.AP(
                            p32.tensor,
                            s_off32 + dy * sv + phi * su + a0 * sa,
                            [[PP32, 128],
                             [sv, K],           # ky
                             [sa, an],          # a
                             [su, CV],          # dx
                             [1, 2]])           # kx pair
                        dst = bass.AP(
                            t32.tensor,
                            phi * pix_w32 + (dy - dy0) * DW,
                            [[t_pitch32, 128],
                             [2, K],                    # ky
                             [4 * pix_w32, an],         # a
                             [K * K // 4, CV],          # dx
                             [1, 2]])                   # kx pair
                        nc.vector.tensor_copy(dst, src)

            A_ST = (NU * A_NJ // 4, A_NJ // 4, 1)
            S_ST = (NU * NJ // 4, NJ // 4, 1)

            def s_off32(t):
                return (S_OFF + t * SEGB) // 4

            def wchunk(t, a0, an, bufs, tag, split=False):
                """Full-depth w-chunk: pixels 4*a0 .. 4*(a0+an)-1 of tile t."""
                T = tp.tile([128, 20 * PIXB], u8, bufs=bufs, name=tag)
                expand(s_off32(t), S_ST, T, 20 * PIXW, PIXW,
                       (0, 1, 2, 3, 4), 0, a0, an)
                nb = 4 * an * PIXB
                if not split:
                    nc.sync.dma_start(
                        bass.AP(out.ap().tensor,
                                t * 128 * OSEG + 4 * a0 * PIXB,
                                [[OSEG, 128], [1, nb]]),
                        T[0:128, 0:nb])
                else:
                    h = nb // 2
                    for i, eng in enumerate((nc.sync, nc.scalar)):
                        eng.dma_start(
                            bass.AP(out.ap().tensor,
                                    t * 128 * OSEG + 4 * a0 * PIXB + i * h,
                                    [[OSEG, 128], [1, h]]),
                            T[0:128, i * h:(i + 1) * h])

            # ---- tile 0 fill --------------------------------------------
            # g1: dy{0,1} x px 0-7 from the fast-start slice (640 B descs)
            Tg1 = tp.tile([128, 8 * 640], u8, bufs=1)
            expand(S0A_OFF // 4, A_ST, Tg1, 8 * 160, 2 * DW, (0, 1), 0,
                   0, 2)
            nc.sync.dma_start(
                bass.AP(out.ap().tensor, 0,
                        [[OSEG, 128], [PIXB, 8], [1, 640]]),
                Tg1[0:128, 0:5120])
            # g2: dy{2,3,4} x px 0-7, also from the slice (960 B descs)
            Tg2 = tp.tile([128, 8 * 960], u8, bufs=1)
            expand(S0A_OFF // 4, A_ST, Tg2, 8 * 240, 3 * DW, (2, 3, 4), 2,
                   0, 2)
            nc.sync.dma_start(
                bass.AP(out.ap().tensor, 640,
                        [[OSEG, 128], [PIXB, 8], [1, 960]]),
                Tg2[0:128, 0:7680])
            # g3/g4: px 8-23, 24-39 full-depth w-chunks
            wchunk(0, 2, 4, 5, "Tst", split=SPLIT_QUEUES)
            wchunk(0, 6, 4, 5, "Tst", split=SPLIT_QUEUES)

            # ---- steady tiles 1,2: 20px w-chunks ------------------------
            for t in (1, 2):
                build_s(t, "even")
                for ch in range(2):
                    wchunk(t, 5 * ch, 5, 5, "Tst", split=SPLIT_QUEUES)
    nc.compile()
    return nc


def _numpy_fallback(x, sr):
    cv = 2 * sr + 1
    off = MAX_SR - sr
    P = np.pad(x[:, 0], ((0, 0), (TP, TP), (TP, TP))).astype(np.uint8)
    out = np.empty((B, H, W, cv * cv, K * K), np.uint8)
    for dy in range(cv):
        for dx in range(cv):
            for ky in range(K):
                for kx in range(K):
                    out[:, :, :, dy * cv + dx, ky * K + kx] = \
                        P[:, off + dy + ky:off + dy + ky + H,
                          off + dx + kx:off + dx + kx + W]
    return out


def kernel(inputs, search_range):
    from concourse.bass_utils import run_bass_kernel_spmd

    x = np.asarray(inputs, dtype=np.float32)
    sr = int(np.asarray(search_range))
    if sr != 2 or x.shape != (B, 1, H, W):
        return _numpy_fallback(x, sr)

    if sr not in _PROG_CACHE:
        _PROG_CACHE[sr] = _build_program(sr)
    nc = _PROG_CACHE[sr]

    host = _make_host_arrays(x, sr)
    res = run_bass_kernel_spmd(nc, host, list(range(NCORES)))
    outs = [np.asarray(res.results[c]["out"]) for c in range(NCORES)]
    return np.concatenate(outs).reshape(B, H, W, CV * CV, K * K)


# revision 24
# speedup vs baseline: 1.0206x; 1.0206x over previous
"""ExtractSearchWindows Trainium2 kernel (8 NeuronCores, Bass/Tile).

out[b, h, w, dy*cv+dx, ky*8+kx] = uint8(P[b, h+off+dy+ky, w+off+dx+kx])
with P = zero-pad(inputs[:, 0], 7) and off = 3 - search_range.

The output (196.6 MB u8) is a pure byte-replication of a tiny input, so
the kernel is bound by per-core DMA-engine write bandwidth (~425 GB/s
across 16 engines; ~26.6 GB/s/engine for descriptors >= 4 KB, less for
small ones).  Work is sharded over (b, h): each of the 8 cores produces
48 output rows as 384 segments (segment = 40-pixel row chunk) in 3
tiles of 128 partitions.

Device-side expansion: strided uint32 DVE tensor_copies read host-
prepared byte-shifted sub-rows S[seg][v][u][j] (v = dy+ky source row,
u = phi+dx byte shift, j = 4a+4kxp+beta addressing pixel w = 4a+phi,
kx = 4*kxp+beta) and scatter them into out-staging tiles that DMA out
with large contiguous descriptors.

Pipeline fill: a small fast-start slice S0a is DMA'd first so the DVE
starts ~1 us earlier; pixels 0-11 of tile 0 drain via two dy-sliced
blocks (640/960 B descriptors, ~0.73x engine rate -- paid while the
engines would otherwise idle); everything later uses w-chunks with
19.2-32 KB descriptors at full rate, sized so the engines never
starve once the first block lands.
"""
import numpy as np

K = 8
MAX_SR = 3
B, H, W = 2, 192, 320
TP = MAX_SR + K // 2          # 7 pad per side
PW = W + 2 * TP               # 334
NCORES = 8
ROWS_PER_CORE = (B * H) // NCORES   # 48
WSEG = 40
NWSEG = W // WSEG             # 8
NSEG = ROWS_PER_CORE * NWSEG  # 384
NTILE = NSEG // 128           # 3

# sr=2 geometry
CV = 5
OSEG = WSEG * CV * CV * K * K   # 64000 output bytes per segment
PIXB = CV * CV * K * K          # 1600 output bytes per pixel
PIXW = PIXB // 4                # 400 u32 per pixel
DW = CV * K * K // 4            # 80 u32 per (pixel, dy)

NV = 12                       # source rows per segment (CV-1+K)
NU = 8                        # byte shifts u = phi+dx
NJ = 44                       # shifted sub-row bytes
SEGB = NV * NU * NJ           # 4224 S bytes per segment
A_NV, A_NJ = 12, 16           # fast-start slice: all v, j<=15 (a<=1)
A_B = A_NV * NU * A_NJ        # 1536
RJ = 56                       # compact row bytes (covers u+j <= 50)
RB = NV * RJ                  # 672 compact bytes per segment

# persistent SBUF layout (u8 offsets)
S0A_OFF = 0
S_OFF = A_B                   # S tiles at S_OFF + t*SEGB
R12_OFF = S_OFF + NTILE * SEGB
PERS_B = R12_OFF + 2 * RB

import os
SPLIT_QUEUES = os.environ.get("ESW_SPLIT_QUEUES", "0") == "1"

_PROG_CACHE = {}


def _make_host_arrays(x, sr):
    """x: (B,1,H,W) f32 -> per-core dict of host-prepped u8 arrays."""
    off = MAX_SR - sr
    P = np.pad(x[:, 0], ((0, 0), (TP, TP), (TP, TP))).astype(np.uint8)
    cores = []
    st = np.lib.stride_tricks.as_strided
    for c in range(NCORES):
        b = (c * ROWS_PER_CORE) // H
        h0 = (c * ROWS_PER_CORE) % H
        flat = np.ascontiguousarray(P[b]).reshape(-1)
        base = (h0 + off) * PW + off
        # S: tile-0 segments fully shifted: (r, s, v, u, j)
        s = st(flat[base:], shape=(16, NWSEG, NV, NU, NJ),
               strides=(PW, WSEG, PW, 1, 1))
        s = np.ascontiguousarray(s).reshape(128, SEGB)
        # S0a: fast-start slice of tile 0 (all v, j<16)
        s0a = st(flat[base:], shape=(16, NWSEG, A_NV, NU, A_NJ),
                 strides=(PW, WSEG, PW, 1, 1))
        s0a = np.ascontiguousarray(s0a).reshape(128, A_B)
        # R12: compact un-shifted rows for tiles 1,2: (t, r, s, v, j)
        r12 = st(flat[base + 16 * PW:], shape=(2, 16, NWSEG, NV, RJ),
                 strides=(16 * PW, PW, WSEG, PW, 1))
        r12 = np.ascontiguousarray(r12.transpose(1, 2, 0, 3, 4)) \
            .reshape(128, 2 * RB)
        cores.append({"s0a": s0a, "s": s, "r12": r12})
    return cores


def _build_program(sr):
    import concourse.bass as bass
    import concourse.bacc as bacc
    import concourse.mybir as mybir
    from concourse import tile

    u8 = mybir.dt.uint8
    u16 = mybir.dt.uint16
    u32 = mybir.dt.uint32
    nc = bacc.Bacc("TRN2", debug=False)
    s0a_in = nc.declare_dram_parameter("s0a", [128, A_B], u8, isOutput=False)
    s_in = nc.declare_dram_parameter("s", [128, SEGB], u8, isOutput=False)
    r12_in = nc.declare_dram_parameter("r12", [128, 2 * RB], u8,
                                       isOutput=False)
    out = nc.declare_dram_parameter("out", [NSEG * OSEG], u8, isOutput=True)

    with tile.TileContext(nc) as tc:
        with tc.tile_pool(name="spool", bufs=1) as sp, \
             tc.tile_pool(name="tpool", bufs=1) as tp:
            PS = sp.tile([128, PERS_B], u8)
            p8 = PS[:]
            p16 = PS[:].bitcast(u16)
            p32 = PS[:].bitcast(u32)
            PP8, PP16, PP32 = PERS_B, PERS_B // 2, PERS_B // 4

            # host data in, latency-critical first, all on the SP queue
            nc.sync.dma_start(PS[:, S0A_OFF:S0A_OFF + A_B], s0a_in[:, :])
            nc.sync.dma_start(PS[:, R12_OFF:R12_OFF + 2 * RB], r12_in[:, :])
            nc.sync.dma_start(PS[:, S_OFF:S_OFF + SEGB], s_in[:, :])

            def build_s(t, parts):
                """Shift compact rows R into S[t]: S[t][v][u][j] =
                R[t][v][u+j].  u%4==0 as u32, u%2==0 as u16 on DVE;
                odd u as u8 on the Activation engine."""
                rb8 = R12_OFF + (t - 1) * RB
                s8 = S_OFF + t * SEGB
                for u in range(NU):
                    if u % 2 == 0 and parts == "even":
                        if u % 4 == 0:
                            src = bass.AP(p32.tensor, rb8 // 4 + u // 4,
                                          [[PP32, 128], [RJ // 4, NV],
                                           [1, NJ // 4]])
                            dst = bass.AP(p32.tensor,
                                          s8 // 4 + u * (NJ // 4),
                                          [[PP32, 128], [NU * NJ // 4, NV],
                                           [1, NJ // 4]])
                            nc.vector.tensor_copy(dst, src)
                        else:
                            src = bass.AP(p16.tensor, rb8 // 2 + u // 2,
                                          [[PP16, 128], [RJ // 2, NV],
                                           [1, NJ // 2]])
                            dst = bass.AP(p16.tensor,
                                          s8 // 2 + u * (NJ // 2),
                                          [[PP16, 128], [NU * NJ // 2, NV],
                                           [1, NJ // 2]])
                            nc.vector.tensor_copy(dst, src)
                    elif u % 2 == 1 and parts == "odd":
                        src = bass.AP(p8.tensor, rb8 + u,
                                      [[PP8, 128], [RJ, NV], [1, NJ]])
                        dst = bass.AP(p8.tensor, s8 + u * NJ,
                                      [[PP8, 128], [NU * NJ, NV], [1, NJ]])
                        nc.vector.tensor_copy(dst, src)



            def expand(s_off32, src_st, T, t_pitch32, pix_w32, dys, dy0,
                       a0, an):
                """DVE scatter block: one copy per (dy in dys, phi 0..3).

                Reads S at u32 offset s_off32 (+ dy*sv + phi*su + a*sa),
                writes staging tile T laid out [pixel][dy-dy0][dx][ky][kx]
                with pix_w32 u32 per pixel.
                """
                sv, su, sa = src_st
                t32 = T[:].bitcast(u32)
                for dy in dys:
                    for phi in range(4):
                        src = bass.AP(
                            p32.tensor,
                            s_off32 + dy * sv + phi * su + a0 * sa,
                            [[PP32, 128],
                             [sv, K],           # ky
                             [sa, an],          # a
                             [su, CV],          # dx
                             [1, 2]])           # kx pair
                        dst = bass.AP(
                            t32.tensor,
                            phi * pix_w32 + (dy - dy0) * DW,
                            [[t_pitch32, 128],
                             [2, K],                    # ky
                             [4 * pix_w32, an],         # a
                             [K * K // 4, CV],          # dx
                             [1, 2]])                   # kx pair
                        nc.vector.tensor_copy(dst, src)

            A_ST = (NU * A_NJ // 4, A_NJ // 4, 1)
            S_ST = (NU * NJ // 4, NJ // 4, 1)

            def s_off32(t):
                return (S_OFF + t * SEGB) // 4

            def wchunk(t, a0, an, bufs, tag, split=False):
                """Full-depth w-chunk: pixels 4*a0 .. 4*(a0+an)-1 of tile t."""
                T = tp.tile([128, 20 * PIXB], u8, bufs=bufs, name=tag)
                expand(s_off32(t), S_ST, T, 20 * PIXW, PIXW,
                       (0, 1, 2, 3, 4), 0, a0, an)
                nb = 4 * an * PIXB
                if not split:
                    nc.sync.dma_start(
                        bass.AP(out.ap().tensor,
                                t * 128 * OSEG + 4 * a0 * PIXB,
                                [[OSEG, 128], [1, nb]]),
                        T[0:128, 0:nb])
                else:
                    h = nb // 2
                    for i, eng in enumerate((nc.sync, nc.scalar)):
                        eng.dma_start(
                            bass.AP(out.ap().tensor,
                                    t * 128 * OSEG + 4 * a0 * PIXB + i * h,
                                    [[OSEG, 128], [1, h]]),
                            T[0:128, i * h:(i + 1) * h])

            # ---- tile 0 fill --------------------------------------------
            # g1: dy{0,1} x px 0-7 from the fast-start slice (640 B descs)
            Tg1 = tp.tile([128, 8 * 640], u8, bufs=1)
            expand(S0A_OFF // 4, A_ST, Tg1, 8 * 160, 2 * DW, (0, 1), 0,
                   0, 2)
            nc.sync.dma_start(
                bass.AP(out.ap().tensor, 0,
                        [[OSEG, 128], [PIXB, 8], [1, 640]]),
                Tg1[0:128, 0:5120])
            # g2: dy{2,3,4} x px 0-7, also from the slice (960 B descs)
            Tg2 = tp.tile([128, 8 * 960], u8, bufs=1)
            expand(S0A_OFF // 4, A_ST, Tg2, 8 * 240, 3 * DW, (2, 3, 4), 2,
                   0, 2)
            nc.sync.dma_start(
                bass.AP(out.ap().tensor, 640,
                        [[OSEG, 128], [PIXB, 8], [1, 960]]),
                Tg2[0:128, 0:7680])
            # g3/g4: px 8-23, 24-39 full-depth w-chunks
            wchunk(0, 2, 4, 5, "Tst", split=SPLIT_QUEUES)
            wchunk(0, 6, 4, 5, "Tst", split=SPLIT_QUEUES)

            # ---- steady tiles 1,2: 20px w-chunks ------------------------
            for t in (1, 2):
                build_s(t, "even")
                build_s(t, "odd")
                for ch in range(2):
                    wchunk(t, 5 * ch, 5, 5, "Tst", split=SPLIT_QUEUES)
    nc.compile()
    return nc


def _numpy_fallback(x, sr):
    cv = 2 * sr + 1
    off = MAX_SR - sr
    P = np.pad(x[:, 0], ((0, 0), (TP, TP), (TP, TP))).astype(np.uint8)
    out = np.empty((B, H, W, cv * cv, K * K), np.uint8)
    for dy in range(cv):
        for dx in range(cv):
            for ky in range(K):
                for kx in range(K):
                    out[:, :, :, dy * cv + dx, ky * K + kx] = \
                        P[:, off + dy + ky:off + dy + ky + H,
                          off + dx + kx:off + dx + kx + W]
    return out


def kernel(inputs, search_range):
    from concourse.bass_utils import run_bass_kernel_spmd

    x = np.asarray(inputs, dtype=np.float32)
    sr = int(np.asarray(search_range))
    if sr != 2 or x.shape != (B, 1, H, W):
        return _numpy_fallback(x, sr)

    if sr not in _PROG_CACHE:
        _PROG_CACHE[sr] = _build_program(sr)
    nc = _PROG_CACHE[sr]

    host = _make_host_arrays(x, sr)
    res = run_bass_kernel_spmd(nc, host, list(range(NCORES)))
    outs = [np.asarray(res.results[c]["out"]) for c in range(NCORES)]
    return np.concatenate(outs).reshape(B, H, W, CV * CV, K * K)


# revision 26
# speedup vs baseline: 1.0445x; 1.0235x over previous
"""ExtractSearchWindows Trainium2 kernel (8 NeuronCores, Bass/Tile).

out[b, h, w, dy*cv+dx, ky*8+kx] = uint8(P[b, h+off+dy+ky, w+off+dx+kx])
with P = zero-pad(inputs[:, 0], 7) and off = 3 - search_range.

The output (196.6 MB u8) is a pure byte-replication of a tiny input, so
the kernel is bound by per-core DMA-engine write bandwidth (~425 GB/s
across 16 engines; ~26.6 GB/s/engine for descriptors >= 4 KB, less for
small ones).  Work is sharded over (b, h): each of the 8 cores produces
48 output rows as 384 segments (segment = 40-pixel row chunk) in 3
tiles of 128 partitions.

Device-side expansion: strided uint32 DVE tensor_copies read host-
prepared byte-shifted sub-rows S[seg][v][u][j] (v = dy+ky source row,
u = phi+dx byte shift, j = 4a+4kxp+beta addressing pixel w = 4a+phi,
kx = 4*kxp+beta) and scatter them into out-staging tiles that DMA out
with large contiguous descriptors.

Pipeline fill: a small fast-start slice S0a is DMA'd first so the DVE
starts ~1 us earlier; pixels 0-11 of tile 0 drain via two dy-sliced
blocks (640/960 B descriptors, ~0.73x engine rate -- paid while the
engines would otherwise idle); everything later uses w-chunks with
19.2-32 KB descriptors at full rate, sized so the engines never
starve once the first block lands.
"""
import numpy as np

K = 8
MAX_SR = 3
B, H, W = 2, 192, 320
TP = MAX_SR + K // 2          # 7 pad per side
PW = W + 2 * TP               # 334
NCORES = 8
ROWS_PER_CORE = (B * H) // NCORES   # 48
WSEG = 40
NWSEG = W // WSEG             # 8
NSEG = ROWS_PER_CORE * NWSEG  # 384
NTILE = NSEG // 128           # 3

# sr=2 geometry
CV = 5
OSEG = WSEG * CV * CV * K * K   # 64000 output bytes per segment
PIXB = CV * CV * K * K          # 1600 output bytes per pixel
PIXW = PIXB // 4                # 400 u32 per pixel
DW = CV * K * K // 4            # 80 u32 per (pixel, dy)

NV = 12                       # source rows per segment (CV-1+K)
NU = 8                        # byte shifts u = phi+dx
NJ = 44                       # shifted sub-row bytes
SEGB = NV * NU * NJ           # 4224 S bytes per segment
A_NV, A_NJ = 12, 16           # fast-start slice: all v, j<=15 (a<=1)
A_B = A_NV * NU * A_NJ        # 1536
RJ = 56                       # compact row bytes (covers u+j <= 50)
RB = NV * RJ                  # 672 compact bytes per segment

# persistent SBUF layout (u8 offsets)
S0A_OFF = 0
S_OFF = A_B                   # S tiles at S_OFF + t*SEGB
R12_OFF = S_OFF + NTILE * SEGB
PERS_B = R12_OFF + 2 * RB

import os
SPLIT_QUEUES = os.environ.get("ESW_SPLIT_QUEUES", "0") == "1"

_PROG_CACHE = {}


def _make_host_arrays(x, sr):
    """x: (B,1,H,W) f32 -> per-core dict of host-prepped u8 arrays."""
    off = MAX_SR - sr
    P = np.pad(x[:, 0], ((0, 0), (TP, TP), (TP, TP))).astype(np.uint8)
    cores = []
    st = np.lib.stride_tricks.as_strided
    for c in range(NCORES):
        b = (c * ROWS_PER_CORE) // H
        h0 = (c * ROWS_PER_CORE) % H
        flat = np.ascontiguousarray(P[b]).reshape(-1)
        base = (h0 + off) * PW + off
        # S: tile-0 segments fully shifted: (r, s, v, u, j)
        s = st(flat[base:], shape=(16, NWSEG, NV, NU, NJ),
               strides=(PW, WSEG, PW, 1, 1))
        s = np.ascontiguousarray(s).reshape(128, SEGB)
        # S0a: fast-start slice of tile 0 (all v, j<16)
        s0a = st(flat[base:], shape=(16, NWSEG, A_NV, NU, A_NJ),
                 strides=(PW, WSEG, PW, 1, 1))
        s0a = np.ascontiguousarray(s0a).reshape(128, A_B)
        # R12: compact un-shifted rows for tiles 1,2: (t, r, s, v, j)
        r12 = st(flat[base + 16 * PW:], shape=(2, 16, NWSEG, NV, RJ),
                 strides=(16 * PW, PW, WSEG, PW, 1))
        r12 = np.ascontiguousarray(r12.transpose(1, 2, 0, 3, 4)) \
            .reshape(128, 2 * RB)
        cores.append({"s0a": s0a, "s": s, "r12": r12})
    return cores


def _build_program(sr):
    import concourse.bass as bass
    import concourse.bacc as bacc
    import concourse.mybir as mybir
    from concourse import tile

    u8 = mybir.dt.uint8
    u16 = mybir.dt.uint16
    u32 = mybir.dt.uint32
    nc = bacc.Bacc("TRN2", debug=False)
    s0a_in = nc.declare_dram_parameter("s0a", [128, A_B], u8, isOutput=False)
    s_in = nc.declare_dram_parameter("s", [128, SEGB], u8, isOutput=False)
    r12_in = nc.declare_dram_parameter("r12", [128, 2 * RB], u8,
                                       isOutput=False)
    out = nc.declare_dram_parameter("out", [NSEG * OSEG], u8, isOutput=True)

    with tile.TileContext(nc) as tc:
        with tc.tile_pool(name="spool", bufs=1) as sp, \
             tc.tile_pool(name="tpool", bufs=1) as tp:
            PS = sp.tile([128, PERS_B], u8)
            p8 = PS[:]
            p16 = PS[:].bitcast(u16)
            p32 = PS[:].bitcast(u32)
            PP8, PP16, PP32 = PERS_B, PERS_B // 2, PERS_B // 4

            # host data in, latency-critical first, all on the SP queue
            nc.sync.dma_start(PS[:, S0A_OFF:S0A_OFF + A_B], s0a_in[:, :])
            nc.sync.dma_start(PS[:, R12_OFF:R12_OFF + 2 * RB], r12_in[:, :])
            nc.sync.dma_start(PS[:, S_OFF:S_OFF + SEGB], s_in[:, :])

            def build_s(t, parts):
                """Shift compact rows R into S[t]: S[t][v][u][j] =
                R[t][v][u+j].  Even u (u32/u16) on DVE, odd u (byte
                shifts) on the otherwise-idle Activation engine."""
                rb8 = R12_OFF + (t - 1) * RB
                s8 = S_OFF + t * SEGB
                for u in range(NU):
                    if u % 2 == 0 and parts == "even":
                        if u % 4 == 0:
                            src_ = bass.AP(p32.tensor, rb8 // 4 + u // 4,
                                           [[PP32, 128], [RJ // 4, NV],
                                            [1, NJ // 4]])
                            dst_ = bass.AP(p32.tensor,
                                           s8 // 4 + u * (NJ // 4),
                                           [[PP32, 128], [NU * NJ // 4, NV],
                                            [1, NJ // 4]])
                            nc.vector.tensor_copy(dst_, src_)
                        else:
                            src_ = bass.AP(p16.tensor, rb8 // 2 + u // 2,
                                           [[PP16, 128], [RJ // 2, NV],
                                            [1, NJ // 2]])
                            dst_ = bass.AP(p16.tensor,
                                           s8 // 2 + u * (NJ // 2),
                                           [[PP16, 128], [NU * NJ // 2, NV],
                                            [1, NJ // 2]])
                            nc.vector.tensor_copy(dst_, src_)
                    elif u % 2 == 1 and parts == "odd":
                        src_ = bass.AP(p8.tensor, rb8 + u,
                                       [[PP8, 128], [RJ, NV], [1, NJ]])
                        dst_ = bass.AP(p8.tensor, s8 + u * NJ,
                                       [[PP8, 128], [NU * NJ, NV], [1, NJ]])
                        nc.scalar.copy(dst_, src_)

            # odd-byte shifts start as soon as R12 lands
            build_s(1, "odd")
            build_s(2, "odd")

            def expand(s_off32, src_st, T, t_pitch32, pix_w32, dys, dy0,
                       a0, an):
                """DVE scatter block: one copy per (dy in dys, phi 0..3).

                Reads S at u32 offset s_off32 (+ dy*sv + phi*su + a*sa),
                writes staging tile T laid out [pixel][dy-dy0][dx][ky][kx]
                with pix_w32 u32 per pixel.
                """
                sv, su, sa = src_st
                t32 = T[:].bitcast(u32)
                for dy in dys:
                    for phi in range(4):
                        src = bass.AP(
                            p32.tensor,
                            s_off32 + dy * sv + phi * su + a0 * sa,
                            [[PP32, 128],
                             [sv, K],           # ky
                             [sa, an],          # a
                             [su, CV],          # dx
                             [1, 2]])           # kx pair
                        dst = bass.AP(
                            t32.tensor,
                            phi * pix_w32 + (dy - dy0) * DW,
                            [[t_pitch32, 128],
                             [2, K],                    # ky
                             [4 * pix_w32, an],         # a
                             [K * K // 4, CV],          # dx
                             [1, 2]])                   # kx pair
                        nc.vector.tensor_copy(dst, src)

            A_ST = (NU * A_NJ // 4, A_NJ // 4, 1)
            S_ST = (NU * NJ // 4, NJ // 4, 1)

            def s_off32(t):
                return (S_OFF + t * SEGB) // 4

            def wchunk(t, a0, an, bufs, tag, split=False):
                """Full-depth w-chunk: pixels 4*a0 .. 4*(a0+an)-1 of tile t."""
                T = tp.tile([128, 20 * PIXB], u8, bufs=bufs, name=tag)
                expand(s_off32(t), S_ST, T, 20 * PIXW, PIXW,
                       (0, 1, 2, 3, 4), 0, a0, an)
                nb = 4 * an * PIXB
                if not split:
                    nc.sync.dma_start(
                        bass.AP(out.ap().tensor,
                                t * 128 * OSEG + 4 * a0 * PIXB,
                                [[OSEG, 128], [1, nb]]),
                        T[0:128, 0:nb])
                else:
                    h = nb // 2
                    for i, eng in enumerate((nc.sync, nc.scalar)):
                        eng.dma_start(
                            bass.AP(out.ap().tensor,
                                    t * 128 * OSEG + 4 * a0 * PIXB + i * h,
                                    [[OSEG, 128], [1, h]]),
                            T[0:128, i * h:(i + 1) * h])

            # ---- tile 0 fill --------------------------------------------
            # g1: dy{0,1} x px 0-7 from the fast-start slice (640 B descs)
            Tg1 = tp.tile([128, 8 * 640], u8, bufs=1)
            expand(S0A_OFF // 4, A_ST, Tg1, 8 * 160, 2 * DW, (0, 1), 0,
                   0, 2)
            nc.sync.dma_start(
                bass.AP(out.ap().tensor, 0,
                        [[OSEG, 128], [PIXB, 8], [1, 640]]),
                Tg1[0:128, 0:5120])
            # g2: dy{2,3,4} x px 0-7, also from the slice (960 B descs)
            Tg2 = tp.tile([128, 8 * 960], u8, bufs=1)
            expand(S0A_OFF // 4, A_ST, Tg2, 8 * 240, 3 * DW, (2, 3, 4), 2,
                   0, 2)
            nc.sync.dma_start(
                bass.AP(out.ap().tensor, 640,
                        [[OSEG, 128], [PIXB, 8], [1, 960]]),
                Tg2[0:128, 0:7680])
            # g3/g4: px 8-23, 24-39 full-depth w-chunks
            wchunk(0, 2, 4, 5, "Tst", split=SPLIT_QUEUES)
            wchunk(0, 6, 4, 5, "Tst", split=SPLIT_QUEUES)

            # ---- steady tiles 1,2: 20px w-chunks ------------------------
            for t in (1, 2):
                build_s(t, "even")
                for ch in range(2):
                    wchunk(t, 5 * ch, 5, 5, "Tst", split=SPLIT_QUEUES)
    nc.compile()
    return nc


def _numpy_fallback(x, sr):
    cv = 2 * sr + 1
    off = MAX_SR - sr
    P = np.pad(x[:, 0], ((0, 0), (TP, TP), (TP, TP))).astype(np.uint8)
    out = np.empty((B, H, W, cv * cv, K * K), np.uint8)
    for dy in range(cv):
        for dx in range(cv):
            for ky in range(K):
                for kx in range(K):
                    out[:, :, :, dy * cv + dx, ky * K + kx] = \
                        P[:, off + dy + ky:off + dy + ky + H,
                          off + dx + kx:off + dx + kx + W]
    return out


def kernel(inputs, search_range):
    from concourse.bass_utils import run_bass_kernel_spmd

    x = np.asarray(inputs, dtype=np.float32)
    sr = int(np.asarray(search_range))
    if sr != 2 or x.shape != (B, 1, H, W):
        return _numpy_fallback(x, sr)

    if sr not in _PROG_CACHE:
        _PROG_CACHE[sr] = _build_program(sr)
    nc = _PROG_CACHE[sr]

    host = _make_host_arrays(x, sr)
    res = run_bass_kernel_spmd(nc, host, list(range(NCORES)))
    outs = [np.asarray(res.results[c]["out"]) for c in range(NCORES)]
    return np.concatenate(outs).reshape(B, H, W, CV * CV, K * K)


# revision 27
# speedup vs baseline: 1.0508x; 1.0060x over previous
"""ExtractSearchWindows Trainium2 kernel (8 NeuronCores, Bass/Tile).

out[b, h, w, dy*cv+dx, ky*8+kx] = uint8(P[b, h+off+dy+ky, w+off+dx+kx])
with P = zero-pad(inputs[:, 0], 7) and off = 3 - search_range.

The output (196.6 MB u8) is a pure byte-replication of a tiny input, so
the kernel is bound by per-core DMA-engine write bandwidth (~425 GB/s
across 16 engines; ~26.6 GB/s/engine for descriptors >= 4 KB, less for
small ones).  Work is sharded over (b, h): each of the 8 cores produces
48 output rows as 384 segments (segment = 40-pixel row chunk) in 3
tiles of 128 partitions.

Device-side expansion: strided uint32 DVE tensor_copies read host-
prepared byte-shifted sub-rows S[seg][v][u][j] (v = dy+ky source row,
u = phi+dx byte shift, j = 4a+4kxp+beta addressing pixel w = 4a+phi,
kx = 4*kxp+beta) and scatter them into out-staging tiles that DMA out
with large contiguous descriptors.

Pipeline fill: a small fast-start slice S0a is DMA'd first so the DVE
starts ~1 us earlier; pixels 0-11 of tile 0 drain via two dy-sliced
blocks (640/960 B descriptors, ~0.73x engine rate -- paid while the
engines would otherwise idle); everything later uses w-chunks with
19.2-32 KB descriptors at full rate, sized so the engines never
starve once the first block lands.
"""
import numpy as np

K = 8
MAX_SR = 3
B, H, W = 2, 192, 320
TP = MAX_SR + K // 2          # 7 pad per side
PW = W + 2 * TP               # 334
NCORES = 8
ROWS_PER_CORE = (B * H) // NCORES   # 48
WSEG = 40
NWSEG = W // WSEG             # 8
NSEG = ROWS_PER_CORE * NWSEG  # 384
NTILE = NSEG // 128           # 3

# sr=2 geometry
CV = 5
OSEG = WSEG * CV * CV * K * K   # 64000 output bytes per segment
PIXB = CV * CV * K * K          # 1600 output bytes per pixel
PIXW = PIXB // 4                # 400 u32 per pixel
DW = CV * K * K // 4            # 80 u32 per (pixel, dy)

NV = 12                       # source rows per segment (CV-1+K)
NU = 8                        # byte shifts u = phi+dx
NJ = 44                       # shifted sub-row bytes
SEGB = NV * NU * NJ           # 4224 S bytes per segment
A_NV, A_NJ = 12, 16           # fast-start slice: all v, j<=15 (a<=1)
A_B = A_NV * NU * A_NJ        # 1536
RJ = 56                       # compact row bytes (covers u+j <= 50)
RB = NV * RJ                  # 672 compact bytes per segment

# persistent SBUF layout (u8 offsets)
S0A_OFF = 0
S_OFF = A_B                   # S tiles at S_OFF + t*SEGB
R12_OFF = S_OFF + NTILE * SEGB
PERS_B = R12_OFF + 2 * RB

import os
SPLIT_QUEUES = os.environ.get("ESW_SPLIT_QUEUES", "0") == "1"

_PROG_CACHE = {}


def _make_host_arrays(x, sr):
    """x: (B,1,H,W) f32 -> per-core dict of host-prepped u8 arrays."""
    off = MAX_SR - sr
    P = np.pad(x[:, 0], ((0, 0), (TP, TP), (TP, TP))).astype(np.uint8)
    cores = []
    st = np.lib.stride_tricks.as_strided
    for c in range(NCORES):
        b = (c * ROWS_PER_CORE) // H
        h0 = (c * ROWS_PER_CORE) % H
        flat = np.ascontiguousarray(P[b]).reshape(-1)
        base = (h0 + off) * PW + off
        # S: tile-0 segments fully shifted: (r, s, v, u, j)
        s = st(flat[base:], shape=(16, NWSEG, NV, NU, NJ),
               strides=(PW, WSEG, PW, 1, 1))
        s = np.ascontiguousarray(s).reshape(128, SEGB)
        # S0a: fast-start slice of tile 0 (all v, j<16)
        s0a = st(flat[base:], shape=(16, NWSEG, A_NV, NU, A_NJ),
                 strides=(PW, WSEG, PW, 1, 1))
        s0a = np.ascontiguousarray(s0a).reshape(128, A_B)
        # R12: compact un-shifted rows for tiles 1,2: (t, r, s, v, j)
        r12 = st(flat[base + 16 * PW:], shape=(2, 16, NWSEG, NV, RJ),
                 strides=(16 * PW, PW, WSEG, PW, 1))
        r12 = np.ascontiguousarray(r12.transpose(1, 2, 0, 3, 4)) \
            .reshape(128, 2 * RB)
        cores.append({"s0a": s0a, "s": s, "r12": r12})
    return cores


def _build_program(sr):
    import concourse.bass as bass
    import concourse.bacc as bacc
    import concourse.mybir as mybir
    from concourse import tile

    u8 = mybir.dt.uint8
    u16 = mybir.dt.uint16
    u32 = mybir.dt.uint32
    nc = bacc.Bacc("TRN2", debug=False)
    s0a_in = nc.declare_dram_parameter("s0a", [128, A_B], u8, isOutput=False)
    s_in = nc.declare_dram_parameter("s", [128, SEGB], u8, isOutput=False)
    r12_in = nc.declare_dram_parameter("r12", [128, 2 * RB], u8,
                                       isOutput=False)
    out = nc.declare_dram_parameter("out", [NSEG * OSEG], u8, isOutput=True)

    with tile.TileContext(nc) as tc:
        with tc.tile_pool(name="spool", bufs=1) as sp, \
             tc.tile_pool(name="tpool", bufs=1) as tp:
            PS = sp.tile([128, PERS_B], u8)
            p8 = PS[:]
            p16 = PS[:].bitcast(u16)
            p32 = PS[:].bitcast(u32)
            PP8, PP16, PP32 = PERS_B, PERS_B // 2, PERS_B // 4

            # host data in, latency-critical first, all on the SP queue
            nc.sync.dma_start(PS[:, S0A_OFF:S0A_OFF + A_B], s0a_in[:, :])
            nc.sync.dma_start(PS[:, R12_OFF:R12_OFF + 2 * RB], r12_in[:, :])
            nc.sync.dma_start(PS[:, S_OFF:S_OFF + SEGB], s_in[:, :])

            def build_s(t, parts):
                """Shift compact rows R into S[t]: S[t][v][u][j] =
                R[t][v][u+j].  Even u (u32/u16) on DVE, odd u (byte
                shifts) on the otherwise-idle Activation engine."""
                rb8 = R12_OFF + (t - 1) * RB
                s8 = S_OFF + t * SEGB
                for u in range(NU):
                    if u % 2 == 0 and parts == "even":
                        if u % 4 == 0:
                            src_ = bass.AP(p32.tensor, rb8 // 4 + u // 4,
                                           [[PP32, 128], [RJ // 4, NV],
                                            [1, NJ // 4]])
                            dst_ = bass.AP(p32.tensor,
                                           s8 // 4 + u * (NJ // 4),
                                           [[PP32, 128], [NU * NJ // 4, NV],
                                            [1, NJ // 4]])
                            nc.vector.tensor_copy(dst_, src_)
                        else:
                            src_ = bass.AP(p16.tensor, rb8 // 2 + u // 2,
                                           [[PP16, 128], [RJ // 2, NV],
                                            [1, NJ // 2]])
                            dst_ = bass.AP(p16.tensor,
                                           s8 // 2 + u * (NJ // 2),
                                           [[PP16, 128], [NU * NJ // 2, NV],
                                            [1, NJ // 2]])
                            nc.vector.tensor_copy(dst_, src_)
                    elif u % 2 == 1 and parts == "odd":
                        src_ = bass.AP(p8.tensor, rb8 + u,
                                       [[PP8, 128], [RJ, NV], [1, NJ]])
                        dst_ = bass.AP(p8.tensor, s8 + u * NJ,
                                       [[PP8, 128], [NU * NJ, NV], [1, NJ]])
                        nc.scalar.copy(dst_, src_)

            # odd-byte shifts start as soon as R12 lands
            build_s(1, "odd")
            build_s(2, "odd")

            def expand(s_off32, src_st, T, t_pitch32, pix_w32, dys, dy0,
                       a0, an):
                """DVE scatter block: one copy per (dy in dys, phi 0..3).

                Reads S at u32 offset s_off32 (+ dy*sv + phi*su + a*sa),
                writes staging tile T laid out [pixel][dy-dy0][dx][ky][kx]
                with pix_w32 u32 per pixel.
                """
                sv, su, sa = src_st
                t32 = T[:].bitcast(u32)
                for dy in dys:
                    for phi in range(4):
                        src = bass.AP(
                            p32.tensor,
                            s_off32 + dy * sv + phi * su + a0 * sa,
                            [[PP32, 128],
                             [sv, K],           # ky
                             [sa, an],          # a
                             [su, CV],          # dx
                             [1, 2]])           # kx pair
                        dst = bass.AP(
                            t32.tensor,
                            phi * pix_w32 + (dy - dy0) * DW,
                            [[t_pitch32, 128],
                             [2, K],                    # ky
                             [4 * pix_w32, an],         # a
                             [K * K // 4, CV],          # dx
                             [1, 2]])                   # kx pair
                        nc.vector.tensor_copy(dst, src)

            A_ST = (NU * A_NJ // 4, A_NJ // 4, 1)
            S_ST = (NU * NJ // 4, NJ // 4, 1)

            def s_off32(t):
                return (S_OFF + t * SEGB) // 4

            def wchunk(t, a0, an, bufs, tag, split=False):
                """Full-depth w-chunk: pixels 4*a0 .. 4*(a0+an)-1 of tile t."""
                T = tp.tile([128, 20 * PIXB], u8, bufs=bufs, name=tag)
                expand(s_off32(t), S_ST, T, 20 * PIXW, PIXW,
                       (0, 1, 2, 3, 4), 0, a0, an)
                nb = 4 * an * PIXB
                if not split:
                    nc.sync.dma_start(
                        bass.AP(out.ap().tensor,
                                t * 128 * OSEG + 4 * a0 * PIXB,
                                [[OSEG, 128], [1, nb]]),
                        T[0:128, 0:nb])
                else:
                    h = nb // 2
                    for i, eng in enumerate((nc.sync, nc.scalar)):
                        eng.dma_start(
                            bass.AP(out.ap().tensor,
                                    t * 128 * OSEG + 4 * a0 * PIXB + i * h,
                                    [[OSEG, 128], [1, h]]),
                            T[0:128, i * h:(i + 1) * h])

            # ---- tile 0 fill --------------------------------------------
            # g1: dy{0,1} x px 0-11 from the fast-start slice (640 B descs)
            Tg1 = tp.tile([128, 12 * 640], u8, bufs=1)
            expand(S0A_OFF // 4, A_ST, Tg1, 12 * 160, 2 * DW, (0, 1), 0,
                   0, 3)
            nc.sync.dma_start(
                bass.AP(out.ap().tensor, 0,
                        [[OSEG, 128], [PIXB, 12], [1, 640]]),
                Tg1[0:128, 0:7680])
            # g2: dy{2,3,4} x px 0-11, also from the slice (960 B descs)
            Tg2 = tp.tile([128, 12 * 960], u8, bufs=1)
            expand(S0A_OFF // 4, A_ST, Tg2, 12 * 240, 3 * DW, (2, 3, 4), 2,
                   0, 3)
            nc.sync.dma_start(
                bass.AP(out.ap().tensor, 640,
                        [[OSEG, 128], [PIXB, 12], [1, 960]]),
                Tg2[0:128, 0:11520])
            # g3/g4: px 12-27, 28-39 full-depth w-chunks
            wchunk(0, 3, 4, 5, "Tst", split=SPLIT_QUEUES)
            wchunk(0, 7, 3, 5, "Tst", split=SPLIT_QUEUES)

            # ---- steady tiles 1,2: 20px w-chunks ------------------------
            for t in (1, 2):
                build_s(t, "even")
                for ch in range(2):
                    wchunk(t, 5 * ch, 5, 5, "Tst", split=SPLIT_QUEUES)
    nc.compile()
    return nc


def _numpy_fallback(x, sr):
    cv = 2 * sr + 1
    off = MAX_SR - sr
    P = np.pad(x[:, 0], ((0, 0), (TP, TP), (TP, TP))).astype(np.uint8)
    out = np.empty((B, H, W, cv * cv, K * K), np.uint8)
    for dy in range(cv):
        for dx in range(cv):
            for ky in range(K):
                for kx in range(K):
                    out[:, :, :, dy * cv + dx, ky * K + kx] = \
                        P[:, off + dy + ky:off + dy + ky + H,
                          off + dx + kx:off + dx + kx + W]
    return out


def kernel(inputs, search_range):
    from concourse.bass_utils import run_bass_kernel_spmd

    x = np.asarray(inputs, dtype=np.float32)
    sr = int(np.asarray(search_range))
    if sr != 2 or x.shape != (B, 1, H, W):
        return _numpy_fallback(x, sr)

    if sr not in _PROG_CACHE:
        _PROG_CACHE[sr] = _build_program(sr)
    nc = _PROG_CACHE[sr]

    host = _make_host_arrays(x, sr)
    res = run_bass_kernel_spmd(nc, host, list(range(NCORES)))
    outs = [np.asarray(res.results[c]["out"]) for c in range(NCORES)]
    return np.concatenate(outs).reshape(B, H, W, CV * CV, K * K)


# revision 28
# speedup vs baseline: 1.1606x; 1.1045x over previous
"""ExtractSearchWindows Trainium2 kernel (8 NeuronCores, Bass/Tile).

out[b, h, w, dy*cv+dx, ky*8+kx] = uint8(P[b, h+off+dy+ky, w+off+dx+kx])
with P = zero-pad(inputs[:, 0], 7) and off = 3 - search_range.

The output (196.6 MB u8) is a pure byte-replication of a tiny input, so
the kernel is bound by per-core DMA-engine write bandwidth (~425 GB/s
across 16 engines; ~26.6 GB/s/engine for descriptors >= 4 KB, less for
small ones).  Work is sharded over (b, h): each of the 8 cores produces
48 output rows as 384 segments (segment = 40-pixel row chunk) in 3
tiles of 128 partitions.

Device-side expansion: strided uint32 DVE tensor_copies read host-
prepared byte-shifted sub-rows S[seg][v][u][j] (v = dy+ky source row,
u = phi+dx byte shift, j = 4a+4kxp+beta addressing pixel w = 4a+phi,
kx = 4*kxp+beta) and scatter them into out-staging tiles that DMA out
with large contiguous descriptors.

Pipeline fill: a small fast-start slice S0a is DMA'd first so the DVE
starts ~1 us earlier; pixels 0-11 of tile 0 drain via two dy-sliced
blocks (640/960 B descriptors, ~0.73x engine rate -- paid while the
engines would otherwise idle); everything later uses w-chunks with
19.2-32 KB descriptors at full rate, sized so the engines never
starve once the first block lands.
"""
import numpy as np

K = 8
MAX_SR = 3
B, H, W = 2, 192, 320
TP = MAX_SR + K // 2          # 7 pad per side
PW = W + 2 * TP               # 334
NCORES = 8
ROWS_PER_CORE = (B * H) // NCORES   # 48
WSEG = 40
NWSEG = W // WSEG             # 8
NSEG = ROWS_PER_CORE * NWSEG  # 384
NTILE = NSEG // 128           # 3

# sr=2 geometry
CV = 5
OSEG = WSEG * CV * CV * K * K   # 64000 output bytes per segment
PIXB = CV * CV * K * K          # 1600 output bytes per pixel
PIXW = PIXB // 4                # 400 u32 per pixel
DW = CV * K * K // 4            # 80 u32 per (pixel, dy)

NV = 12                       # source rows per segment (CV-1+K)
NU = 8                        # byte shifts u = phi+dx
NJ = 44                       # shifted sub-row bytes
SEGB = NV * NU * NJ           # 4224 S bytes per segment
A_NV, A_NJ = 12, 16           # fast-start slice: all v, j<=15 (a<=1)
A_B = A_NV * NU * A_NJ        # 1536
RJ = 56                       # compact row bytes (covers u+j <= 50)
RB = NV * RJ                  # 672 compact bytes per segment

# persistent SBUF layout (u8 offsets)
S0A_OFF = 0
S_OFF = A_B                   # S tiles at S_OFF + t*SEGB
R12_OFF = S_OFF + NTILE * SEGB
PERS_B = R12_OFF + 2 * RB

import os
SPLIT_QUEUES = os.environ.get("ESW_SPLIT_QUEUES", "0") == "1"

_PROG_CACHE = {}


def _make_host_arrays(x, sr):
    """x: (B,1,H,W) f32 -> per-core dict of host-prepped u8 arrays."""
    off = MAX_SR - sr
    P = np.pad(x[:, 0], ((0, 0), (TP, TP), (TP, TP))).astype(np.uint8)
    cores = []
    st = np.lib.stride_tricks.as_strided
    for c in range(NCORES):
        b = (c * ROWS_PER_CORE) // H
        h0 = (c * ROWS_PER_CORE) % H
        flat = np.ascontiguousarray(P[b]).reshape(-1)
        base = (h0 + off) * PW + off
        # S: tile-0 segments fully shifted: (r, s, v, u, j)
        s = st(flat[base:], shape=(16, NWSEG, NV, NU, NJ),
               strides=(PW, WSEG, PW, 1, 1))
        s = np.ascontiguousarray(s).reshape(128, SEGB)
        # S0a: fast-start slice of tile 0 (all v, j<16)
        s0a = st(flat[base:], shape=(16, NWSEG, A_NV, NU, A_NJ),
                 strides=(PW, WSEG, PW, 1, 1))
        s0a = np.ascontiguousarray(s0a).reshape(128, A_B)
        # R12: compact un-shifted rows for tiles 1,2: (t, r, s, v, j)
        r12 = st(flat[base + 16 * PW:], shape=(2, 16, NWSEG, NV, RJ),
                 strides=(16 * PW, PW, WSEG, PW, 1))
        r12 = np.ascontiguousarray(r12.transpose(1, 2, 0, 3, 4)) \
            .reshape(128, 2 * RB)
        cores.append({"s0a": s0a, "s": s, "r12": r12})
    return cores


def _build_program(sr):
    import concourse.bass as bass
    import concourse.bacc as bacc
    import concourse.mybir as mybir
    from concourse import tile

    u8 = mybir.dt.uint8
    u16 = mybir.dt.uint16
    u32 = mybir.dt.uint32
    nc = bacc.Bacc("TRN2", debug=False)
    s0a_in = nc.declare_dram_parameter("s0a", [128, A_B], u8, isOutput=False)
    s_in = nc.declare_dram_parameter("s", [128, SEGB], u8, isOutput=False)
    r12_in = nc.declare_dram_parameter("r12", [128, 2 * RB], u8,
                                       isOutput=False)
    out = nc.declare_dram_parameter("out", [NSEG * OSEG], u8, isOutput=True)

    with tile.TileContext(nc) as tc:
        with tc.tile_pool(name="spool", bufs=1) as sp, \
             tc.tile_pool(name="tpool", bufs=1) as tp:
            PS = sp.tile([128, PERS_B], u8)
            p8 = PS[:]
            p16 = PS[:].bitcast(u16)
            p32 = PS[:].bitcast(u32)
            PP8, PP16, PP32 = PERS_B, PERS_B // 2, PERS_B // 4

            # warmup: absorb the DMA-engine ramp on throwaway bytes
            Twu = tp.tile([128, 512], u8, bufs=1)
            nc.sync.dma_start(Twu[:, :], s_in[:, 0:512])
            # host data in, latency-critical first, all on the SP queue
            nc.sync.dma_start(PS[:, S0A_OFF:S0A_OFF + A_B], s0a_in[:, :])
            nc.sync.dma_start(PS[:, R12_OFF:R12_OFF + 2 * RB], r12_in[:, :])
            nc.sync.dma_start(PS[:, S_OFF:S_OFF + SEGB], s_in[:, :])

            def build_s(t, parts):
                """Shift compact rows R into S[t]: S[t][v][u][j] =
                R[t][v][u+j].  Even u (u32/u16) on DVE, odd u (byte
                shifts) on the otherwise-idle Activation engine."""
                rb8 = R12_OFF + (t - 1) * RB
                s8 = S_OFF + t * SEGB
                for u in range(NU):
                    if u % 2 == 0 and parts == "even":
                        if u % 4 == 0:
                            src_ = bass.AP(p32.tensor, rb8 // 4 + u // 4,
                                           [[PP32, 128], [RJ // 4, NV],
                                            [1, NJ // 4]])
                            dst_ = bass.AP(p32.tensor,
                                           s8 // 4 + u * (NJ // 4),
                                           [[PP32, 128], [NU * NJ // 4, NV],
                                            [1, NJ // 4]])
                            nc.vector.tensor_copy(dst_, src_)
                        else:
                            src_ = bass.AP(p16.tensor, rb8 // 2 + u // 2,
                                           [[PP16, 128], [RJ // 2, NV],
                                            [1, NJ // 2]])
                            dst_ = bass.AP(p16.tensor,
                                           s8 // 2 + u * (NJ // 2),
                                           [[PP16, 128], [NU * NJ // 2, NV],
                                            [1, NJ // 2]])
                            nc.vector.tensor_copy(dst_, src_)
                    elif u % 2 == 1 and parts == "odd":
                        src_ = bass.AP(p8.tensor, rb8 + u,
                                       [[PP8, 128], [RJ, NV], [1, NJ]])
                        dst_ = bass.AP(p8.tensor, s8 + u * NJ,
                                       [[PP8, 128], [NU * NJ, NV], [1, NJ]])
                        nc.scalar.copy(dst_, src_)

            # odd-byte shifts start as soon as R12 lands
            build_s(1, "odd")
            build_s(2, "odd")

            def expand(s_off32, src_st, T, t_pitch32, pix_w32, dys, dy0,
                       a0, an):
                """DVE scatter block: one copy per (dy in dys, phi 0..3).

                Reads S at u32 offset s_off32 (+ dy*sv + phi*su + a*sa),
                writes staging tile T laid out [pixel][dy-dy0][dx][ky][kx]
                with pix_w32 u32 per pixel.
                """
                sv, su, sa = src_st
                t32 = T[:].bitcast(u32)
                for dy in dys:
                    for phi in range(4):
                        src = bass.AP(
                            p32.tensor,
                            s_off32 + dy * sv + phi * su + a0 * sa,
                            [[PP32, 128],
                             [sv, K],           # ky
                             [sa, an],          # a
                             [su, CV],          # dx
                             [1, 2]])           # kx pair
                        dst = bass.AP(
                            t32.tensor,
                            phi * pix_w32 + (dy - dy0) * DW,
                            [[t_pitch32, 128],
                             [2, K],                    # ky
                             [4 * pix_w32, an],         # a
                             [K * K // 4, CV],          # dx
                             [1, 2]])                   # kx pair
                        nc.vector.tensor_copy(dst, src)

            A_ST = (NU * A_NJ // 4, A_NJ // 4, 1)
            S_ST = (NU * NJ // 4, NJ // 4, 1)

            def s_off32(t):
                return (S_OFF + t * SEGB) // 4

            def wchunk(t, a0, an, bufs, tag, split=False):
                """Full-depth w-chunk: pixels 4*a0 .. 4*(a0+an)-1 of tile t."""
                T = tp.tile([128, 20 * PIXB], u8, bufs=bufs, name=tag)
                expand(s_off32(t), S_ST, T, 20 * PIXW, PIXW,
                       (0, 1, 2, 3, 4), 0, a0, an)
                nb = 4 * an * PIXB
                if not split:
                    nc.sync.dma_start(
                        bass.AP(out.ap().tensor,
                                t * 128 * OSEG + 4 * a0 * PIXB,
                                [[OSEG, 128], [1, nb]]),
                        T[0:128, 0:nb])
                else:
                    h = nb // 2
                    for i, eng in enumerate((nc.sync, nc.scalar)):
                        eng.dma_start(
                            bass.AP(out.ap().tensor,
                                    t * 128 * OSEG + 4 * a0 * PIXB + i * h,
                                    [[OSEG, 128], [1, h]]),
                            T[0:128, i * h:(i + 1) * h])

            # ---- tile 0 fill --------------------------------------------
            # g0: dy{0,1} x px 0-3 from the fast-start slice (640 B descs)
            Tg0 = tp.tile([128, 4 * 640], u8, bufs=1)
            expand(S0A_OFF // 4, A_ST, Tg0, 4 * 160, 2 * DW, (0, 1), 0,
                   0, 1)
            nc.sync.dma_start(
                bass.AP(out.ap().tensor, 0,
                        [[OSEG, 128], [PIXB, 4], [1, 640]]),
                Tg0[0:128, 0:2560])
            # g1: dy{0,1} x px 4-11
            Tg1 = tp.tile([128, 8 * 640], u8, bufs=1)
            expand(S0A_OFF // 4, A_ST, Tg1, 8 * 160, 2 * DW, (0, 1), 0,
                   1, 2)
            nc.sync.dma_start(
                bass.AP(out.ap().tensor, 4 * PIXB,
                        [[OSEG, 128], [PIXB, 8], [1, 640]]),
                Tg1[0:128, 0:5120])
            # g2: dy{2,3,4} x px 0-11, also from the slice (960 B descs)
            Tg2 = tp.tile([128, 12 * 960], u8, bufs=1)
            expand(S0A_OFF // 4, A_ST, Tg2, 12 * 240, 3 * DW, (2, 3, 4), 2,
                   0, 3)
            nc.sync.dma_start(
                bass.AP(out.ap().tensor, 640,
                        [[OSEG, 128], [PIXB, 12], [1, 960]]),
                Tg2[0:128, 0:11520])
            # g3/g4: px 12-27, 28-39 full-depth w-chunks
            wchunk(0, 3, 4, 5, "Tst", split=SPLIT_QUEUES)
            wchunk(0, 7, 3, 5, "Tst", split=SPLIT_QUEUES)

            # ---- steady tiles 1,2: 20px w-chunks ------------------------
            for t in (1, 2):
                build_s(t, "even")
                for ch in range(2):
                    wchunk(t, 5 * ch, 5, 5, "Tst", split=SPLIT_QUEUES)
    nc.compile()
    return nc


def _numpy_fallback(x, sr):
    cv = 2 * sr + 1
    off = MAX_SR - sr
    P = np.pad(x[:, 0], ((0, 0), (TP, TP), (TP, TP))).astype(np.uint8)
    out = np.empty((B, H, W, cv * cv, K * K), np.uint8)
    for dy in range(cv):
        for dx in range(cv):
            for ky in range(K):
                for kx in range(K):
                    out[:, :, :, dy * cv + dx, ky * K + kx] = \
                        P[:, off + dy + ky:off + dy + ky + H,
                          off + dx + kx:off + dx + kx + W]
    return out


def kernel(inputs, search_range):
    from concourse.bass_utils import run_bass_kernel_spmd

    x = np.asarray(inputs, dtype=np.float32)
    sr = int(np.asarray(search_range))
    if sr != 2 or x.shape != (B, 1, H, W):
        return _numpy_fallback(x, sr)

    if sr not in _PROG_CACHE:
        _PROG_CACHE[sr] = _build_program(sr)
    nc = _PROG_CACHE[sr]

    host = _make_host_arrays(x, sr)
    res = run_bass_kernel_spmd(nc, host, list(range(NCORES)))
    outs = [np.asarray(res.results[c]["out"]) for c in range(NCORES)]
    return np.concatenate(outs).reshape(B, H, W, CV * CV, K * K)


# revision 29
# speedup vs baseline: 1.2339x; 1.0632x over previous
"""ExtractSearchWindows Trainium2 kernel (8 NeuronCores, Bass/Tile).

out[b, h, w, dy*cv+dx, ky*8+kx] = uint8(P[b, h+off+dy+ky, w+off+dx+kx])
with P = zero-pad(inputs[:, 0], 7) and off = 3 - search_range.

The output (196.6 MB u8) is a pure byte-replication of a tiny input, so
the kernel is bound by per-core DMA-engine write bandwidth (~425 GB/s
across 16 engines; ~26.6 GB/s/engine for descriptors >= 4 KB, less for
small ones).  Work is sharded over (b, h): each of the 8 cores produces
48 output rows as 384 segments (segment = 40-pixel row chunk) in 3
tiles of 128 partitions.

Device-side expansion: strided uint32 DVE tensor_copies read host-
prepared byte-shifted sub-rows S[seg][v][u][j] (v = dy+ky source row,
u = phi+dx byte shift, j = 4a+4kxp+beta addressing pixel w = 4a+phi,
kx = 4*kxp+beta) and scatter them into out-staging tiles that DMA out
with large contiguous descriptors.

Pipeline fill: a small fast-start slice S0a is DMA'd first so the DVE
starts ~1 us earlier; pixels 0-11 of tile 0 drain via two dy-sliced
blocks (640/960 B descriptors, ~0.73x engine rate -- paid while the
engines would otherwise idle); everything later uses w-chunks with
19.2-32 KB descriptors at full rate, sized so the engines never
starve once the first block lands.
"""
import numpy as np

K = 8
MAX_SR = 3
B, H, W = 2, 192, 320
TP = MAX_SR + K // 2          # 7 pad per side
PW = W + 2 * TP               # 334
NCORES = 8
ROWS_PER_CORE = (B * H) // NCORES   # 48
WSEG = 40
NWSEG = W // WSEG             # 8
NSEG = ROWS_PER_CORE * NWSEG  # 384
NTILE = NSEG // 128           # 3

# sr=2 geometry
CV = 5
OSEG = WSEG * CV * CV * K * K   # 64000 output bytes per segment
PIXB = CV * CV * K * K          # 1600 output bytes per pixel
PIXW = PIXB // 4                # 400 u32 per pixel
DW = CV * K * K // 4            # 80 u32 per (pixel, dy)

NV = 12                       # source rows per segment (CV-1+K)
NU = 8                        # byte shifts u = phi+dx
NJ = 44                       # shifted sub-row bytes
SEGB = NV * NU * NJ           # 4224 S bytes per segment
A_NV, A_NJ = 12, 16           # fast-start slice: all v, j<=15 (a<=1)
A_B = A_NV * NU * A_NJ        # 1536
RJ = 56                       # compact row bytes (covers u+j <= 50)
RB = NV * RJ                  # 672 compact bytes per segment

# persistent SBUF layout (u8 offsets)
S0A_OFF = 0
S_OFF = A_B                   # S tiles at S_OFF + t*SEGB
R12_OFF = S_OFF + NTILE * SEGB
PERS_B = R12_OFF + 2 * RB

import os
SPLIT_QUEUES = os.environ.get("ESW_SPLIT_QUEUES", "0") == "1"

_PROG_CACHE = {}


def _make_host_arrays(x, sr):
    """x: (B,1,H,W) f32 -> per-core dict of host-prepped u8 arrays."""
    off = MAX_SR - sr
    P = np.pad(x[:, 0], ((0, 0), (TP, TP), (TP, TP))).astype(np.uint8)
    cores = []
    st = np.lib.stride_tricks.as_strided
    for c in range(NCORES):
        b = (c * ROWS_PER_CORE) // H
        h0 = (c * ROWS_PER_CORE) % H
        flat = np.ascontiguousarray(P[b]).reshape(-1)
        base = (h0 + off) * PW + off
        # S: tile-0 segments fully shifted: (r, s, v, u, j)
        s = st(flat[base:], shape=(16, NWSEG, NV, NU, NJ),
               strides=(PW, WSEG, PW, 1, 1))
        s = np.ascontiguousarray(s).reshape(128, SEGB)
        # S0a: fast-start slice of tile 0 (all v, j<16)
        s0a = st(flat[base:], shape=(16, NWSEG, A_NV, NU, A_NJ),
                 strides=(PW, WSEG, PW, 1, 1))
        s0a = np.ascontiguousarray(s0a).reshape(128, A_B)
        # R12: compact un-shifted rows for tiles 1,2: (t, r, s, v, j)
        r12 = st(flat[base + 16 * PW:], shape=(2, 16, NWSEG, NV, RJ),
                 strides=(16 * PW, PW, WSEG, PW, 1))
        r12 = np.ascontiguousarray(r12.transpose(1, 2, 0, 3, 4)) \
            .reshape(128, 2 * RB)
        cores.append({"s0a": s0a, "s": s, "r12": r12})
    return cores


def _build_program(sr):
    import concourse.bass as bass
    import concourse.bacc as bacc
    import concourse.mybir as mybir
    from concourse import tile

    u8 = mybir.dt.uint8
    u16 = mybir.dt.uint16
    u32 = mybir.dt.uint32
    nc = bacc.Bacc("TRN2", debug=False)
    s0a_in = nc.declare_dram_parameter("s0a", [128, A_B], u8, isOutput=False)
    s_in = nc.declare_dram_parameter("s", [128, SEGB], u8, isOutput=False)
    r12_in = nc.declare_dram_parameter("r12", [128, 2 * RB], u8,
                                       isOutput=False)
    out = nc.declare_dram_parameter("out", [NSEG * OSEG], u8, isOutput=True)

    with tile.TileContext(nc) as tc:
        with tc.tile_pool(name="spool", bufs=1) as sp, \
             tc.tile_pool(name="tpool", bufs=1) as tp:
            PS = sp.tile([128, PERS_B], u8)
            p8 = PS[:]
            p16 = PS[:].bitcast(u16)
            p32 = PS[:].bitcast(u32)
            PP8, PP16, PP32 = PERS_B, PERS_B // 2, PERS_B // 4

            # host data in, latency-critical first, all on the SP queue
            nc.sync.dma_start(PS[:, S0A_OFF:S0A_OFF + A_B], s0a_in[:, :])
            nc.sync.dma_start(PS[:, R12_OFF:R12_OFF + 2 * RB], r12_in[:, :])
            nc.sync.dma_start(PS[:, S_OFF:S_OFF + SEGB], s_in[:, :])

            def build_s(t, parts):
                """Shift compact rows R into S[t]: S[t][v][u][j] =
                R[t][v][u+j].  Even u (u32/u16) on DVE, odd u (byte
                shifts) on the otherwise-idle Activation engine."""
                rb8 = R12_OFF + (t - 1) * RB
                s8 = S_OFF + t * SEGB
                for u in range(NU):
                    if u % 2 == 0 and parts == "even":
                        if u % 4 == 0:
                            src_ = bass.AP(p32.tensor, rb8 // 4 + u // 4,
                                           [[PP32, 128], [RJ // 4, NV],
                                            [1, NJ // 4]])
                            dst_ = bass.AP(p32.tensor,
                                           s8 // 4 + u * (NJ // 4),
                                           [[PP32, 128], [NU * NJ // 4, NV],
                                            [1, NJ // 4]])
                            nc.vector.tensor_copy(dst_, src_)
                        else:
                            src_ = bass.AP(p16.tensor, rb8 // 2 + u // 2,
                                           [[PP16, 128], [RJ // 2, NV],
                                            [1, NJ // 2]])
                            dst_ = bass.AP(p16.tensor,
                                           s8 // 2 + u * (NJ // 2),
                                           [[PP16, 128], [NU * NJ // 2, NV],
                                            [1, NJ // 2]])
                            nc.vector.tensor_copy(dst_, src_)
                    elif u % 2 == 1 and parts == "odd":
                        src_ = bass.AP(p8.tensor, rb8 + u,
                                       [[PP8, 128], [RJ, NV], [1, NJ]])
                        dst_ = bass.AP(p8.tensor, s8 + u * NJ,
                                       [[PP8, 128], [NU * NJ, NV], [1, NJ]])
                        nc.scalar.copy(dst_, src_)

            # odd-byte shifts start as soon as R12 lands
            build_s(1, "odd")
            build_s(2, "odd")

            def expand(s_off32, src_st, T, t_pitch32, pix_w32, dys, dy0,
                       a0, an):
                """DVE scatter block: one copy per (dy in dys, phi 0..3).

                Reads S at u32 offset s_off32 (+ dy*sv + phi*su + a*sa),
                writes staging tile T laid out [pixel][dy-dy0][dx][ky][kx]
                with pix_w32 u32 per pixel.
                """
                sv, su, sa = src_st
                t32 = T[:].bitcast(u32)
                for dy in dys:
                    for phi in range(4):
                        src = bass.AP(
                            p32.tensor,
                            s_off32 + dy * sv + phi * su + a0 * sa,
                            [[PP32, 128],
                             [sv, K],           # ky
                             [sa, an],          # a
                             [su, CV],          # dx
                             [1, 2]])           # kx pair
                        dst = bass.AP(
                            t32.tensor,
                            phi * pix_w32 + (dy - dy0) * DW,
                            [[t_pitch32, 128],
                             [2, K],                    # ky
                             [4 * pix_w32, an],         # a
                             [K * K // 4, CV],          # dx
                             [1, 2]])                   # kx pair
                        nc.vector.tensor_copy(dst, src)

            A_ST = (NU * A_NJ // 4, A_NJ // 4, 1)
            S_ST = (NU * NJ // 4, NJ // 4, 1)

            def s_off32(t):
                return (S_OFF + t * SEGB) // 4

            def wchunk(t, a0, an, bufs, tag, split=False):
                """Full-depth w-chunk: pixels 4*a0 .. 4*(a0+an)-1 of tile t."""
                T = tp.tile([128, 20 * PIXB], u8, bufs=bufs, name=tag)
                expand(s_off32(t), S_ST, T, 20 * PIXW, PIXW,
                       (0, 1, 2, 3, 4), 0, a0, an)
                nb = 4 * an * PIXB
                if not split:
                    nc.sync.dma_start(
                        bass.AP(out.ap().tensor,
                                t * 128 * OSEG + 4 * a0 * PIXB,
                                [[OSEG, 128], [1, nb]]),
                        T[0:128, 0:nb])
                else:
                    h = nb // 2
                    for i, eng in enumerate((nc.sync, nc.scalar)):
                        eng.dma_start(
                            bass.AP(out.ap().tensor,
                                    t * 128 * OSEG + 4 * a0 * PIXB + i * h,
                                    [[OSEG, 128], [1, h]]),
                            T[0:128, i * h:(i + 1) * h])

            # ---- tile 0 fill --------------------------------------------
            # g1: dy{0,1} x px 0-11 from the fast-start slice (640 B descs)
            Tg1 = tp.tile([128, 12 * 640], u8, bufs=1)
            expand(S0A_OFF // 4, A_ST, Tg1, 12 * 160, 2 * DW, (0, 1), 0,
                   0, 3)
            nc.sync.dma_start(
                bass.AP(out.ap().tensor, 0,
                        [[OSEG, 128], [PIXB, 12], [1, 640]]),
                Tg1[0:128, 0:7680])
            # g2: dy{2,3,4} x px 0-11, also from the slice (960 B descs)
            Tg2 = tp.tile([128, 12 * 960], u8, bufs=1)
            expand(S0A_OFF // 4, A_ST, Tg2, 12 * 240, 3 * DW, (2, 3, 4), 2,
                   0, 3)
            nc.sync.dma_start(
                bass.AP(out.ap().tensor, 640,
                        [[OSEG, 128], [PIXB, 12], [1, 960]]),
                Tg2[0:128, 0:11520])
            # g3/g4: px 12-27, 28-39 full-depth w-chunks
            wchunk(0, 3, 4, 5, "Tst", split=SPLIT_QUEUES)
            wchunk(0, 7, 3, 5, "Tst", split=SPLIT_QUEUES)

            # ---- steady tiles 1,2: 20px w-chunks ------------------------
            for t in (1, 2):
                build_s(t, "even")
                for ch in range(2):
                    wchunk(t, 5 * ch, 5, 5, "Tst", split=SPLIT_QUEUES)
    nc.compile()
    return nc


def _numpy_fallback(x, sr):
    cv = 2 * sr + 1
    off = MAX_SR - sr
    P = np.pad(x[:, 0], ((0, 0), (TP, TP), (TP, TP))).astype(np.uint8)
    out = np.empty((B, H, W, cv * cv, K * K), np.uint8)
    for dy in range(cv):
        for dx in range(cv):
            for ky in range(K):
                for kx in range(K):
                    out[:, :, :, dy * cv + dx, ky * K + kx] = \
                        P[:, off + dy + ky:off + dy + ky + H,
                          off + dx + kx:off + dx + kx + W]
    return out


def kernel(inputs, search_range):
    from concourse.bass_utils import run_bass_kernel_spmd

    x = np.asarray(inputs, dtype=np.float32)
    sr = int(np.asarray(search_range))
    if sr != 2 or x.shape != (B, 1, H, W):
        return _numpy_fallback(x, sr)

    if sr not in _PROG_CACHE:
        _PROG_CACHE[sr] = _build_program(sr)
    nc = _PROG_CACHE[sr]

    host = _make_host_arrays(x, sr)
    res = run_bass_kernel_spmd(nc, host, list(range(NCORES)))
    outs = [np.asarray(res.results[c]["out"]) for c in range(NCORES)]
    return np.concatenate(outs).reshape(B, H, W, CV * CV, K * K)
